# revision 6
# baseline (speedup 1.0000x reference)
"""Trainium2 Bass kernel for BoundaryCorrectionModule — mixed bf16/fp8-DoubleRow.

Full inputs in, full output out; pure data-parallel over batch on 8 cores
(2048 rows each), 4 sub-passes of 512 rows per core. Activations live
feature-major [128p, kc, batch] in SBUF.

Precision: per-unit choice between bf16 matmuls and fp8(e4m3) DoubleRow
matmuls (2x PE throughput), selected offline by per-unit error-sensitivity
knapsack so the final rel err stays < 2e-2 (sim-validated 1.74e-2, HW
baseline bf16 err is 5.4e-3). fp8 weights are quantized at grid x1024;
bf16 weights are pre-scaled by 1024 exactly, so any mix of fp8/bf16 units
can accumulate into one PSUM group and share a single x(1/1024) descale
applied by the ACT evacuation. Activations are quantized 1:1 (values are
O(1)); dual bf16+fp8 copies are kept for tensors consumed by both kinds of
units (fp8 copies made by spare ACT-engine Copy ops).

Step-0 algebraic fold: S0 = 0.5*(h_prev+h_next), so step 0 uses
W_SM' = W_SM + 0.5*W_SS, W_SD' = W_SD + 0.5*W_SS and never computes S0.
"""

import numpy as np
import ml_dtypes

import concourse.bass as bass
import concourse.mybir as mybir
import concourse.tile as tile
from concourse import bacc
from concourse.bass_utils import run_bass_kernel_spmd

BF16 = ml_dtypes.bfloat16
F8 = ml_dtypes.float8_e4m3     # IEEE e4m3: max 240 == TRN FP8_EXP4
F32 = np.float32

B = 16384
D = 1024
NCORES = 8
BC = B // NCORES               # 2048 rows per core
NHALF = 4                      # sub-passes per core
H = BC // NHALF                # 512 rows per pass
FREE = 512                     # PSUM bank / moving free dim
KC = D // 128                  # 8 contraction chunks
MC = D // 128                  # 8 output-feature chunks
NSTEP = 2
WSCALE = 1024.0                # fp8 weight grid scale (power of 2)
DESCALE = float(1.0 / WSCALE)

# fp8 weight units (e4m3, scaled x1024), packed [MC, 128, KC, 128]
W8_KEYS = ["gS_M", "gS_D", "SS", "SD", "Mz_S", "Mz_D", "Mr_S", "Mr_M",
           "Mr_D", "Mh_S", "gD_S", "gD_M", "DS", "DM", "DD"]
# bf16 weight units (pre-scaled x1024), packed [MC, 128, KC, 128]
WB_KEYS = ["SM0", "SD0", "SM", "Mz_M", "Mz_D", "Mh_S", "Mh_M", "Mh_D",
           "DM", "DD"]
W8 = {k: i for i, k in enumerate(W8_KEYS)}
WB = {k: i for i, k in enumerate(WB_KEYS)}

B_gS, B_Mz, B_Mr, B_Mh, B_gD = 0, 1, 2, 3, 4
NBIAS = 5

SIG = mybir.ActivationFunctionType.Sigmoid
TANH = mybir.ActivationFunctionType.Tanh
COPY = mybir.ActivationFunctionType.Copy

_BUILD_CACHE = {}


def _pack_unit(wt_block, np_dtype, scale):
    """[D(k), D(m)] f32 (already W.T) -> [MC, 128, KC, 128] contiguous."""
    w = np.asarray(wt_block, F32) * F32(scale)
    if np_dtype is F8:
        w = np.clip(w, -240.0, 240.0)
    w = w.reshape(KC, 128, MC, 128)                 # (kc, p, mc, m)
    w = np.transpose(w, (2, 1, 0, 3))               # (mc, p, kc, m)
    return np.ascontiguousarray(w.astype(np_dtype))


def _pack_acts(hT_core, np_dtype):
    """[D, BC] f32 (feature-major, one core) -> [NHALF, 128, KC, H]."""
    x = hT_core.reshape(KC, 128, BC)                # (kc, p, b)
    x = np.transpose(x, (1, 0, 2))                  # (p, kc, b)
    x = x.reshape(128, KC, NHALF, H)
    x = np.transpose(x, (2, 0, 1, 3))               # (h, p, kc, b)
    return np.ascontiguousarray(x.astype(np_dtype))


def _build():
    key = (NHALF, FREE)
    if key in _BUILD_CACHE:
        return _BUILD_CACHE[key]

    nc = bacc.Bacc("TRN2", target_bir_lowering=False, debug=False)
    bf = mybir.dt.bfloat16
    f8 = mybir.dt.float8e4
    f32 = mybir.dt.float32
    DR = mybir.MatmulPerfMode.DoubleRow

    hp_d = nc.dram_tensor("hp", [NHALF, 128, KC, H], bf, kind="ExternalInput")
    hn_d = nc.dram_tensor("hn", [NHALF, 128, KC, H], bf, kind="ExternalInput")
    hp8_d = nc.dram_tensor("hp8", [NHALF, 128, KC, H], f8, kind="ExternalInput")
    hn8_d = nc.dram_tensor("hn8", [NHALF, 128, KC, H], f8, kind="ExternalInput")
    w8_d = nc.dram_tensor("w8", [len(W8_KEYS), MC, 128, KC, 128], f8,
                          kind="ExternalInput")
    wb_d = nc.dram_tensor("wb", [len(WB_KEYS), MC, 128, KC, 128], bf,
                          kind="ExternalInput")
    bias_d = nc.dram_tensor("bias", [128, NBIAS, MC], f32, kind="ExternalInput")
    rs_d = nc.dram_tensor("rs", [128, 1], f32, kind="ExternalInput")
    out_d = nc.dram_tensor("out", [NHALF, MC, 128, H], f32, kind="ExternalOutput")

    with tile.TileContext(nc) as tc:
        with (
            tc.tile_pool(name="const", bufs=1) as const_p,
            tc.tile_pool(name="st", bufs=3) as st_p,       # bf16 states
            tc.tile_pool(name="st8", bufs=3) as st8_p,     # fp8 states
            tc.tile_pool(name="aux", bufs=2) as aux_p,     # DVE-only tensors
            tc.tile_pool(name="w8p", bufs=10) as w8_p,
            tc.tile_pool(name="wbp", bufs=8) as wb_p,
            tc.tile_pool(name="tp", bufs=4) as t_p,
            tc.tile_pool(name="dp", bufs=3) as d_p,
            tc.tile_pool(name="op", bufs=2) as o_p,
            tc.tile_pool(name="ps", bufs=8, space="PSUM") as ps_p,
        ):
            bias_t = const_p.tile([128, NBIAS, MC], f32)
            nc.gpsimd.dma_start(bias_t[:], bias_d.ap()[:, :, :])
            rs_t = const_p.tile([128, 1], f32)
            nc.gpsimd.dma_start(rs_t[:], rs_d.ap()[:, :])

            def load_w8(u, mc, eng=None):
                w = w8_p.tile([128, KC, 128], f8, tag="w8", name=f"w8_{u}_{mc}")
                (eng or nc.sync).dma_start(w[:], w8_d.ap()[W8[u], mc])
                return w

            def load_wb(u, mc, kc0=0):
                nk = KC - kc0
                w = wb_p.tile([128, KC, 128], bf, tag="wb", name=f"wb_{u}_{mc}")
                nc.sync.dma_start(w[:, :nk, :], wb_d.ap()[WB[u], mc, :, kc0:KC, :])
                return w

            def new_t(pool, tag, name, dt, bufs=None):
                return pool.tile([128, KC, FREE], dt, tag=tag, name=name,
                                 uniquify=True, bufs=bufs)

            def phase(units, bias_idx, evac, w_eng=None):
                """units: list of (wkey, (src_bf, src_f8), np8) with np8 the
                number of fp8 kc-pairs (0..4); remaining kc chunks run bf16."""
                for mc in range(MC):
                    wts = []
                    for (wkey, src, np8) in units:
                        w8t = load_w8(wkey, mc, eng=w_eng) if np8 > 0 else None
                        wbt = load_wb(wkey, mc, kc0=2 * np8) if np8 < 4 else None
                        wts.append((w8t, wbt, src, np8))
                    p = ps_p.tile([128, FREE], f32, tag="p", name=f"p{mc}")
                    total = sum(np8 + (KC - 2 * np8) for (_, _, np8) in units)
                    i = 0
                    for (w8t, wbt, (sbf, s8), np8) in wts:
                        for j in range(np8):
                            nc.tensor.matmul(
                                p[:, :], w8t[:, 2 * j:2 * j + 2, :],
                                s8[:, 2 * j:2 * j + 2, :],
                                start=(i == 0), stop=(i == total - 1),
                                perf_mode=DR)
                            i += 1
                        for kc in range(2 * np8, KC):
                            nc.tensor.matmul(
                                p[:, :], wbt[:, kc - 2 * np8, :], sbf[:, kc, :],
                                start=(i == 0), stop=(i == total - 1))
                            i += 1
                    if bias_idx is None:
                        evac(p, mc, 0.0)
                    else:
                        evac(p, mc, bias_t[:, bias_idx, mc:mc + 1])

            def evac_plain(dst, func):
                def f(psum, mc, b_ap):
                    nc.scalar.activation(dst[:, mc, :], psum[:, :], func,
                                         bias=b_ap, scale=DESCALE)
                return f

            def evac_gated(dst_bf, dst_8, func, gate):
                def f(psum, mc, b_ap):
                    t = t_p.tile([128, FREE], bf, tag="t", name=f"t{mc}")
                    nc.scalar.activation(t[:], psum[:, :], func, bias=b_ap,
                                         scale=DESCALE)
                    nc.vector.tensor_mul(dst_bf[:, mc, :], t[:], gate[:, mc, :])
                    if dst_8 is not None:
                        # cast on the DVE: it directly follows its producer in
                        # that queue, so it never blocks the next mc's ACT the
                        # way a scalar-engine Copy would
                        nc.vector.tensor_scalar_add(dst_8[:, mc, :],
                                                    dst_bf[:, mc, :], 0.0)
                return f

            def evac_gru(dst_bf, dst_8, M_old, z):
                def f(psum, mc, b_ap):
                    t = t_p.tile([128, FREE], bf, tag="t", name=f"t{mc}")
                    nc.scalar.activation(t[:], psum[:, :], TANH, bias=b_ap,
                                         scale=DESCALE)
                    d = d_p.tile([128, FREE], bf, tag="d", name=f"d{mc}")
                    nc.vector.tensor_sub(d[:], t[:], M_old[:, mc, :])
                    nc.vector.tensor_mul(d[:], d[:], z[:, mc, :])
                    nc.vector.tensor_add(dst_bf[:, mc, :], M_old[:, mc, :], d[:])
                    if dst_8 is not None:
                        nc.vector.tensor_scalar_add(dst_8[:, mc, :],
                                                    dst_bf[:, mc, :], 0.0)
                return f

            def make_inputs(h):
                M_bf = new_t(st_p, "M", f"M_{h}", bf)
                M_8 = new_t(st8_p, "M8", f"M8_{h}", f8)
                D_bf = new_t(st_p, "D", f"D_{h}", bf)
                D_8 = new_t(st8_p, "D8", f"D8_{h}", f8)
                # input states go on the gpsimd engine's DMA queue: it has no
                # other work, so next-half prefetch issues as soon as the ring
                # frees, and the sync-queue weight stream is never blocked.
                # fp8 copies first — the first phase (gS) consumes them.
                nc.gpsimd.dma_start(M_8[:], hp8_d.ap()[h])
                nc.gpsimd.dma_start(D_8[:], hn8_d.ap()[h])
                nc.gpsimd.dma_start(M_bf[:], hp_d.ap()[h])
                nc.gpsimd.dma_start(D_bf[:], hn_d.ap()[h])
                return (M_bf, M_8), (D_bf, D_8)

            pending = None   # (M, Dv, GS) pipelined from the previous half
            for h in range(NHALF):
                if pending is None:
                    M, Dv = make_inputs(h)
                    GS_pre = None
                else:
                    M, Dv, GS_pre = pending
                    pending = None
                S = None

                for step in range(NSTEP):
                    last = step == NSTEP - 1

                    if step == 0 and GS_pre is not None:
                        GS = GS_pre        # gS phase already ran, pipelined
                    else:
                        GS = new_t(aux_p, "GS", f"GS_{h}_{step}", bf, bufs=1)
                        phase([("gS_M", M, 4), ("gS_D", Dv, 4)], B_gS,
                              evac_plain(GS, SIG))

                    S_bf = new_t(st_p, "S", f"S_{h}_{step}", bf)
                    S_8 = new_t(st8_p, "S8", f"S8_{h}_{step}", f8)
                    if step == 0:
                        s_units = [("SM0", M, 0), ("SD0", Dv, 0)]
                    else:
                        s_units = [("SS", S, 4), ("SM", M, 0), ("SD", Dv, 4)]
                    phase(s_units, None, evac_gated(S_bf, S_8, TANH, GS))
                    S_new = (S_bf, S_8)

                    z = new_t(aux_p, "z", f"z_{h}_{step}", bf)
                    mz = [("Mz_S", S_new, 4), ("Mz_M", M, 0),
                          ("Mz_D", Dv, 4 if step == 1 else 0)]
                    phase(mz, B_Mz, evac_plain(z, SIG))

                    rM_bf = new_t(aux_p, "rM", f"rM_{h}_{step}", bf)
                    phase([("Mr_S", S_new, 4), ("Mr_M", M, 4), ("Mr_D", Dv, 4)],
                          B_Mr, evac_gated(rM_bf, None, SIG, M[0]))
                    rM = (rM_bf, None)

                    Mn_bf = new_t(st_p, "M", f"Mn_{h}_{step}", bf)
                    Mn_8 = new_t(st8_p, "M8", f"Mn8_{h}_{step}", f8)
                    # rM is the freshest input — contract it last
                    mh = [("Mh_S", S_new, 4 if step == 0 else 0),
                          ("Mh_D", Dv, 0), ("Mh_M", rM, 0)]
                    phase(mh, B_Mh, evac_gru(Mn_bf, Mn_8, M[0], z))
                    M_new = (Mn_bf, Mn_8)

                    if step == 1 and h + 1 < NHALF:
                        # software-pipeline the next half: its inputs and gS
                        # phase are independent DR work that keeps the PE busy
                        # while this half's Mh evacuation chain drains.
                        Mn_in, Dn_in = make_inputs(h + 1)
                        GSn = new_t(aux_p, "GS", f"GS_{h + 1}_0", bf, bufs=1)
                        phase([("gS_M", Mn_in, 4), ("gS_D", Dn_in, 4)], B_gS,
                              evac_plain(GSn, SIG))
                        pending = (Mn_in, Dn_in, GSn)

                    GD = new_t(aux_p, "GD", f"GD_{h}_{step}", bf, bufs=1)

                    # gD and D phases fused per mc: all matmuls that do NOT
                    # need M_new's fp8 cast (gD_S, DS, DD) are queued ahead of
                    # the M_new-dependent ones (gD_M, DM), so the PE has work
                    # while the Mh evacuation chain drains.
                    def emit_units(p, wts_sub, i, total):
                        for (w8t, wbt, (sbf, s8), np8) in wts_sub:
                            for j in range(np8):
                                nc.tensor.matmul(
                                    p[:, :], w8t[:, 2 * j:2 * j + 2, :],
                                    s8[:, 2 * j:2 * j + 2, :],
                                    start=(i == 0), stop=(i == total - 1),
                                    perf_mode=DR)
                                i += 1
                            for kc in range(2 * np8, KC):
                                nc.tensor.matmul(
                                    p[:, :], wbt[:, kc - 2 * np8, :],
                                    sbf[:, kc, :],
                                    start=(i == 0), stop=(i == total - 1))
                                i += 1
                        return i

                    def fused_gd_d(evac_d, step=step):
                        dm8 = 4 if step == 1 else 2
                        dd8 = 4 if step == 1 else 0
                        for mc in range(MC):
                            w_gs = (load_w8("gD_S", mc), None, S_new, 4)
                            w_gm = (load_w8("gD_M", mc), None, M_new, 4)
                            w_ds = (load_w8("DS", mc), None, S_new, 4)
                            w_dd = (load_w8("DD", mc) if dd8 else None,
                                    load_wb("DD", mc, kc0=2 * dd8)
                                    if dd8 < 4 else None, Dv, dd8)
                            w_dm = (load_w8("DM", mc),
                                    load_wb("DM", mc, kc0=2 * dm8)
                                    if dm8 < 4 else None, M_new, dm8)
                            pA = ps_p.tile([128, FREE], f32, tag="p",
                                           name=f"pA{mc}")
                            pB = ps_p.tile([128, FREE], f32, tag="p",
                                           name=f"pB{mc}")
                            totA, totB = 8, (4 + dd8 + (KC - 2 * dd8)
                                             + dm8 + (KC - 2 * dm8))
                            iA = emit_units(pA, [w_gs], 0, totA)
                            iB = emit_units(pB, [w_ds, w_dd], 0, totB)
                            emit_units(pA, [w_gm], iA, totA)
                            nc.scalar.activation(GD[:, mc, :], pA[:, :], SIG,
                                                 bias=bias_t[:, B_gD, mc:mc + 1],
                                                 scale=DESCALE)
                            emit_units(pB, [w_dm], iB, totB)
                            evac_d(pB, mc, 0.0)

                    if not last:
                        Dn_bf = new_t(st_p, "D", f"Dn_{h}_{step}", bf)
                        Dn_8 = new_t(st8_p, "D8", f"Dn8_{h}_{step}", f8)
                        fused_gd_d(evac_gated(Dn_bf, Dn_8, TANH, GD))
                        S, M, Dv = S_new, M_new, (Dn_bf, Dn_8)
                    else:
                        def evac_final(psum, mc, b_ap, _S=S_new, _M=M_new,
                                       _GD=GD, _h=h):
                            t = t_p.tile([128, FREE], bf, tag="t", name=f"t{mc}")
                            nc.scalar.activation(t[:], psum[:, :], TANH,
                                                 bias=b_ap, scale=DESCALE)
                            d = d_p.tile([128, FREE], bf, tag="d", name=f"d{mc}")
                            nc.vector.tensor_mul(d[:], t[:], _GD[:, mc, :])
                            o = o_p.tile([128, FREE], f32, tag="o",
                                         name=f"o_{_h}_{mc}")
                            nc.vector.tensor_add(o[:], _S[0][:, mc, :], d[:])
                            nc.vector.tensor_scalar_mul(o[:], o[:], rs_t[:, 0:1])
                            nc.vector.tensor_add(o[:], o[:], _M[0][:, mc, :])
                            nc.sync.dma_start(out_d.ap()[_h, mc], o[:])
                        fused_gd_d(evac_final)

    nc.compile()
    _BUILD_CACHE[key] = nc
    return nc


def _pack_inputs(h_prev, h_next, W_SS, W_SM, W_SD, W_Mz, b_Mz, W_Mr, b_Mr,
                 W_Mh, b_Mh, W_DS, W_DM, W_DD, W_gS, b_gS, W_gD, b_gD,
                 residual_scale):
    f = np.float32

    def T(w):
        return np.ascontiguousarray(np.asarray(w, f).T)

    t_ss, t_sm, t_sd = T(W_SS), T(W_SM), T(W_SD)
    gs, gd = T(W_gS), T(W_gD)
    mz, mr, mh = T(W_Mz), T(W_Mr), T(W_Mh)
    blocks = {
        "SS": t_ss, "SM": t_sm, "SD": t_sd,
        "SM0": t_sm + f(0.5) * t_ss, "SD0": t_sd + f(0.5) * t_ss,
        "gS_M": gs[:D], "gS_D": gs[D:],
        "gD_S": gd[:D], "gD_M": gd[D:],
        "Mz_S": mz[:D], "Mz_M": mz[D:2 * D], "Mz_D": mz[2 * D:],
        "Mr_S": mr[:D], "Mr_M": mr[D:2 * D], "Mr_D": mr[2 * D:],
        "Mh_S": mh[:D], "Mh_M": mh[D:2 * D], "Mh_D": mh[2 * D:],
        "DS": T(W_DS), "DM": T(W_DM), "DD": T(W_DD),
    }
    w8 = np.stack([_pack_unit(blocks[k], F8, WSCALE) for k in W8_KEYS])
    wb = np.stack([_pack_unit(blocks[k], BF16, WSCALE) for k in WB_KEYS])

    bias = np.stack([np.asarray(b, f) for b in (b_gS, b_Mz, b_Mr, b_Mh, b_gD)])
    bias = bias.reshape(NBIAS, MC, 128)
    bias = np.ascontiguousarray(np.transpose(bias, (2, 0, 1)))

    rs = np.full((128, 1), np.asarray(residual_scale, f), dtype=f)

    hpT = np.asarray(h_prev, f).T
    hnT = np.asarray(h_next, f).T

    in_maps = []
    for c in range(NCORES):
        sl = slice(c * BC, (c + 1) * BC)
        hp_c = np.ascontiguousarray(hpT[:, sl])
        hn_c = np.ascontiguousarray(hnT[:, sl])
        in_maps.append({
            "hp": _pack_acts(hp_c, BF16),
            "hn": _pack_acts(hn_c, BF16),
            "hp8": _pack_acts(hp_c, F8),
            "hn8": _pack_acts(hn_c, F8),
            "w8": w8,
            "wb": wb,
            "bias": bias,
            "rs": rs,
        })
    return in_maps


def _unpack_output(results):
    blocks = []
    for c in range(NCORES):
        a = results[c]["out"]                        # [NHALF, MC, 128, H]
        a = np.transpose(a, (1, 2, 0, 3)).reshape(D, BC)
        blocks.append(a)
    outT = np.concatenate(blocks, axis=1)
    return np.ascontiguousarray(outT.T)


def run(trace=False, tmpdir=None, trace_kwargs=None, **inputs):
    nc = _build()
    in_maps = _pack_inputs(**inputs)
    res = run_bass_kernel_spmd(
        nc, in_maps, core_ids=list(range(NCORES)),
        trace=trace, tmpdir=tmpdir, **(trace_kwargs or {}))
    return _unpack_output(res.results), res


def kernel(**inputs):
    import os
    os.environ["BASS_NEVER_TRACE"] = "1"
    try:
        out, _ = run(**inputs)
    finally:
        os.environ.pop("BASS_NEVER_TRACE", None)
    return out
